# revision 6
# baseline (speedup 1.0000x reference)
"""Trainium2 Bass kernel for the 2-layer ARMA GNN (nn_ARMA_30374008717356).

Self-contained: accepts FULL inputs, returns FULL output. Internally:
- 5 SPMD launches on 8 NeuronCores via run_bass_kernel_spmd.
- Math folding: norm = dinv[src]*w*dinv[dst] is folded as
  h~ = (x@W)*dinv (row scale at the table), per-edge scale w only, and a
  dinv[dst] row scale applied after aggregation. So
  agg[d] = dinv[d] * sum_e w_e * h~[src_e].
- Aggregation launches are 2D-sharded: core = (dst half, src quarter), so
  every dma_gather call's int16 indices fit a single 25000-row window.
  Per-core slot grids: dsts sorted by per-core edge count, tiles of 128
  rows x D_pad slot columns; pad slots point at row 0 with weight 0.
- Host between launches does only sharding / permutation / padding
  (index manipulation); all arithmetic happens on device.
"""
import os
import sys
import types
import contextlib
import ctypes

import numpy as np

sys.path.insert(0, "/opt/trn_rl_repo")

import concourse.bass as bass
import concourse.bacc as bacc
import concourse.mybir as mybir
import concourse.tile as tile
from concourse.bass_utils import run_bass_kernel_spmd
from concourse.masks import make_identity

# ---------------------------------------------------------------- constants
N = 100000
E = 1600000
F_IN, HID, NCLS = 64, 48, 40
NC = 8
SH = N // NC          # 12500 nodes per core (layer-compute shards)
HALF = N // 2         # 50000 dst half
QTR = N // 4          # 25000 src quarter window (int16-safe)
P = 128
FP = 64               # padded feature count -> 256B gather rows
GCALL_COLS = 7        # 7 cols * 128 rows = 896 idx per dma_gather call (ring-safe)
SB_COLS = 160         # superblock column budget (~41KB/partition fp32)

F32 = mybir.dt.float32
I16 = mybir.dt.int16

_TRACE = [False]      # set by test harness to collect exec times
_EXEC_NS = []


# ------------------------------------------------------------ axon NTFF shim
def _install_profile_shim():
    if "antenv.axon_hooks" in sys.modules:
        return
    try:
        import antenv
        from trn_agent_boot.trn_boot import _ntff_profile_via_ctypes
    except Exception:
        return
    hook_holder = {"h": None}
    mod = types.ModuleType("antenv.axon_hooks")
    mod.set_axon_ntff_profile_hook = lambda h: hook_holder.__setitem__("h", h)
    mod.get_axon_ntff_profile_hook = lambda: hook_holder["h"]
    sys.modules["antenv.axon_hooks"] = mod
    antenv.axon_hooks = mod
    try:
        h = _ntff_profile_via_ctypes("/opt/axon/libaxon_pjrt.so")
        if h is not None:
            mod.set_axon_ntff_profile_hook(h)
    except Exception:
        pass


_install_profile_shim()


def _run(nc, in_maps):
    trace = _TRACE[0]
    res = run_bass_kernel_spmd(nc, in_maps, core_ids=list(range(NC)), trace=trace)
    if trace:
        _EXEC_NS.append(res.exec_time_ns)
    return res.results


def _new_nc(nq=1):
    return bacc.Bacc(
        "TRN2",
        target_bir_lowering=False,
        debug=False,
        num_devices=NC,
        num_swdge_queues=nq,
    )


# ================================================================ host prep
def _slot_grid(dloc, order_key_counts, ndst, vals_list):
    """Build per-dst slot assignment for one core.

    dloc: local dst id per edge (int64, [M])
    order_key_counts: per-dst edge counts [ndst]
    vals_list: list of per-edge value arrays to place into grids
    Returns (perm, rank_counts_sorted, row, col) where row = rank of dst,
    col = slot index within dst, perm = dst ids sorted by count desc.
    """
    counts = order_key_counts
    perm = np.argsort(-counts, kind="stable")
    rank = np.empty(ndst, np.int64)
    rank[perm] = np.arange(ndst)
    row = rank[dloc]
    order = np.argsort(row, kind="stable")
    row_s = row[order]
    counts_sorted = counts[perm]
    starts = np.zeros(ndst, np.int64)
    np.cumsum(counts_sorted[:-1], out=starts[1:])
    col = np.arange(len(row_s)) - starts[row_s]
    return perm, counts_sorted, row_s, col, order


def _grid_inputs(row_s, col, ntiles, dprof, idx_vals, w_vals):
    """Assemble device inputs for one aggregation core.

    Returns (idx_blocks [128, totcols*? int16 16-wrapped per call],
             w_grid [128, totcols] f32, plan) -- plan computed separately.
    idx_vals/w_vals are in slot order (row_s, col).
    """
    totcols = int(np.sum(dprof))
    col_off = np.zeros(ntiles, np.int64)
    np.cumsum(dprof[:-1], out=col_off[1:])
    tile_of_row = row_s // P
    part = row_s % P
    gcol = col_off[tile_of_row] + col
    wg = np.zeros((P, totcols), np.float32)
    ig = np.zeros((P, totcols), np.int16)
    wg[part, gcol] = w_vals
    ig[part, gcol] = idx_vals
    return ig, wg, totcols


def _wrap_idx_calls(ig, call_plan):
    """ig: [128, totcols] int16 grid. call_plan: list of (c0, ncols).
    Returns [128, sum(ncols*8)] int16 ready for device (16-wrap + x8 tile)."""
    blocks = []
    for c0, ncols in call_plan:
        flat = ig[:, c0:c0 + ncols].T.ravel()          # i = col*128 + p
        b = flat.reshape(-1, 16).T                      # [16, n/16]
        blocks.append(np.tile(b, (8, 1)))               # [128, n/16]
    return np.concatenate(blocks, axis=1)


def _make_call_plan(dprof):
    """Split the flat column space into superblocks and gather calls.

    Returns list of superblocks; each is dict with tiles [(tile_idx, c0_rel,
    D)], ncols, calls [(c0_rel, ncols)], c0_abs."""
    sbs = []
    cur = {"tiles": [], "ncols": 0, "c0_abs": 0}
    c_abs = 0
    for t, d in enumerate(dprof):
        d = int(d)
        if d == 0:
            # still need output; attach to current superblock with D=0
            cur["tiles"].append((t, cur["ncols"], 0))
            continue
        if cur["ncols"] + d > SB_COLS and cur["ncols"] > 0:
            sbs.append(cur)
            cur = {"tiles": [], "ncols": 0, "c0_abs": c_abs}
        cur["tiles"].append((t, cur["ncols"], d))
        cur["ncols"] += d
        c_abs += d
    if cur["tiles"]:
        sbs.append(cur)
    for sb in sbs:
        calls = []
        c = 0
        while c < sb["ncols"]:
            n = min(GCALL_COLS, sb["ncols"] - c)
            calls.append((c, n))
            c += n
        sb["calls"] = calls
    return sbs


# ================================================================ kernels
def _build_k1(d1prof):
    """Per-core: deg -> dinv; h1 = (x@W1)*dinv (padded to 64); s1 = x@V1+b1.

    Inputs: xt [64, SH], w1slots [128, tot1], wcat [65, 96] (=[ [W1|V1]; [0|b1] ]).
    Outputs: h1p [SH,64], s1 [SH,48], dinv [SH,1].
    """
    ntiles = (SH + P - 1) // P
    tot1 = int(np.sum(d1prof))
    off = np.zeros(ntiles, np.int64)
    np.cumsum(d1prof[:-1], out=off[1:])

    nc = _new_nc()
    xt = nc.dram_tensor("xt", [F_IN, SH], F32, kind="ExternalInput")
    wsl = nc.dram_tensor("wsl", [P, max(tot1, 1)], F32, kind="ExternalInput")
    wcat = nc.dram_tensor("wcat", [F_IN, HID * 2], F32, kind="ExternalInput")
    bcat = nc.dram_tensor("bcat", [1, HID * 2], F32, kind="ExternalInput")
    h1p = nc.dram_tensor("h1p", [SH, FP], F32, kind="ExternalOutput")
    s1 = nc.dram_tensor("s1", [SH, HID], F32, kind="ExternalOutput")
    dinv_o = nc.dram_tensor("dinv", [SH, 1], F32, kind="ExternalOutput")

    with tile.TileContext(nc) as tc:
        with (
            tc.tile_pool(name="cst", bufs=1) as cst,
            tc.tile_pool(name="sb", bufs=3) as pool,
            tc.tile_pool(name="ps", bufs=2, space="PSUM") as psp,
        ):
            xt_t = cst.tile([F_IN, SH], F32)
            nc.sync.dma_start(out=xt_t[:], in_=xt[:])
            wsl_t = cst.tile([P, max(tot1, 1)], F32)
            nc.sync.dma_start(out=wsl_t[:], in_=wsl[:])
            wc_t = cst.tile([F_IN, HID * 2], F32)
            nc.sync.dma_start(out=wc_t[:], in_=wcat[:])
            bc_t = cst.tile([1, HID * 2], F32)
            nc.sync.dma_start(out=bc_t[:], in_=bcat[:])
            ones_t = cst.tile([1, P], F32)
            nc.vector.memset(ones_t[:], 1.0)

            for t in range(ntiles):
                rows = min(P, SH - t * P)
                d1 = int(d1prof[t])
                # ---- degree + dinv
                dinv_t = pool.tile([P, 1], F32, tag="dinv")
                if d1 > 0:
                    deg = pool.tile([P, 1], F32, tag="deg")
                    nc.vector.reduce_sum(
                        out=deg[:], in_=wsl_t[:, off[t]:off[t] + d1],
                        axis=mybir.AxisListType.X,
                    )
                    mask = pool.tile([P, 1], F32, tag="mask")
                    nc.vector.tensor_scalar(
                        out=mask[:], in0=deg[:], scalar1=0.0, scalar2=None,
                        op0=mybir.AluOpType.is_gt,
                    )
                    degc = pool.tile([P, 1], F32, tag="degc")
                    nc.vector.tensor_scalar_max(out=degc[:], in0=deg[:], scalar1=1e-30)
                    sq = pool.tile([P, 1], F32, tag="sq")
                    nc.scalar.activation(
                        out=sq[:], in_=degc[:],
                        func=mybir.ActivationFunctionType.Sqrt,
                    )
                    rs = pool.tile([P, 1], F32, tag="rs")
                    nc.vector.reciprocal(out=rs[:], in_=sq[:])
                    nc.vector.tensor_mul(out=dinv_t[:], in0=rs[:], in1=mask[:])
                else:
                    nc.vector.memset(dinv_t[:], 0.0)
                nc.sync.dma_start(
                    out=dinv_o[t * P:t * P + rows, :], in_=dinv_t[:rows, :]
                )
                # ---- matmul [xW | xV] + [0 | b]
                ps = psp.tile([P, HID * 2], F32, tag="mm")
                nc.tensor.matmul(
                    out=ps[:rows, :], lhsT=xt_t[:, t * P:t * P + rows],
                    rhs=wc_t[:, :], start=True, stop=False,
                )
                nc.tensor.matmul(
                    out=ps[:rows, :], lhsT=ones_t[:, :rows],
                    rhs=bc_t[:, :], start=False, stop=True,
                )
                # h1 = xW * dinv, padded to FP cols
                h1t = pool.tile([P, FP], F32, tag="h1")
                nc.vector.memset(h1t[:], 0.0)
                nc.vector.tensor_scalar_mul(
                    out=h1t[:rows, :HID], in0=ps[:rows, :HID], scalar1=dinv_t[:rows, :]
                )
                nc.sync.dma_start(out=h1p[t * P:t * P + rows, :], in_=h1t[:rows, :])
                s1t = pool.tile([P, HID], F32, tag="s1")
                nc.vector.tensor_copy(out=s1t[:rows, :], in_=ps[:rows, HID:])
                nc.sync.dma_start(out=s1[t * P:t * P + rows, :], in_=s1t[:rows, :])
    nc.compile()
    return nc


def _build_k2(dprof, fuse_cols):
    """Aggregation core: partial[r] = sum_slots w * table[idx].

    Inputs: table [QTR, FP], idxw (16-wrapped per call) [128, idx_cols],
    wg [128, totcols]. Output: partial [ntiles*128, fuse_cols].
    """
    ntiles = len(dprof)
    sbs = _make_call_plan(dprof)
    idx_cols = sum(8 * n for sb in sbs for (_, n) in sb["calls"])
    totcols = int(np.sum(dprof))

    nc = _new_nc(nq=4)
    table = nc.dram_tensor("table", [QTR, FP], F32, kind="ExternalInput")
    idxw = nc.dram_tensor("idxw", [P, max(idx_cols, 1)], I16, kind="ExternalInput")
    wg = nc.dram_tensor("wg", [P, max(totcols, 1)], F32, kind="ExternalInput")
    partial = nc.dram_tensor("partial", [ntiles * P, fuse_cols], F32, kind="ExternalOutput")

    qn = [0]

    with tile.TileContext(nc) as tc:
        with (
            tc.tile_pool(name="cst", bufs=1) as cst,
            tc.tile_pool(name="gat", bufs=2) as gat,
            tc.tile_pool(name="wrk", bufs=3) as wrk,
        ):
            idx_t = cst.tile([P, max(idx_cols, 1)], I16)
            nc.sync.dma_start(out=idx_t[:], in_=idxw[:])
            wg_t = cst.tile([P, max(totcols, 1)], F32)
            nc.sync.dma_start(out=wg_t[:], in_=wg[:])

            iwoff = 0
            for sb in sbs:
                ncols = sb["ncols"]
                if ncols > 0:
                    g = gat.tile([P, ncols, FP], F32, tag="g")
                    for (c0, cn) in sb["calls"]:
                        nidx = P * cn
                        nc.gpsimd.dma_gather(
                            g[:, c0:c0 + cn, :],
                            table[:],
                            idx_t[:, iwoff:iwoff + 8 * cn],
                            nidx,
                            nidx,
                            FP,
                            queue_num=qn[0] % 4,
                        )
                        qn[0] += 1
                        iwoff += 8 * cn
                for (t, c0r, d) in sb["tiles"]:
                    out_t = wrk.tile([P, fuse_cols], F32, tag="out")
                    if d == 0:
                        nc.vector.memset(out_t[:], 0.0)
                    else:
                        c_abs = sb["c0_abs"] + c0r
                        sc = wrk.tile([P, d, fuse_cols], F32, tag="sc")
                        nc.vector.tensor_mul(
                            out=sc[:],
                            in0=g[:, c0r:c0r + d, :fuse_cols],
                            in1=wg_t[:, c_abs:c_abs + d]
                                .to_broadcast([P, d, fuse_cols]),
                        )
                        nc.vector.reduce_sum(
                            out=out_t[:],
                            in_=sc[:].rearrange("p d f -> p f d"),
                            axis=mybir.AxisListType.X,
                        )
                    nc.sync.dma_start(
                        out=partial[t * P:(t + 1) * P, :], in_=out_t[:]
                    )
    nc.compile()
    return nc


def _build_k3():
    """Combine 4 partials, finish layer 1, produce layer-2 table inputs.

    out1 = relu(dinv * (p0+p1+p2+p3) + s1)
    h2p = pad64((out1 @ W2) * dinv); s2 = out1 @ V2 + b2.
    """
    ntiles = (SH + P - 1) // P
    nc = _new_nc()
    ps_ = [nc.dram_tensor(f"p{i}", [SH, HID], F32, kind="ExternalInput") for i in range(4)]
    s1 = nc.dram_tensor("s1", [SH, HID], F32, kind="ExternalInput")
    dinv = nc.dram_tensor("dinv", [SH, 1], F32, kind="ExternalInput")
    w2c = nc.dram_tensor("w2c", [HID, NCLS * 2], F32, kind="ExternalInput")
    b2c = nc.dram_tensor("b2c", [1, NCLS * 2], F32, kind="ExternalInput")
    h2p = nc.dram_tensor("h2p", [SH, FP], F32, kind="ExternalOutput")
    s2 = nc.dram_tensor("s2", [SH, NCLS], F32, kind="ExternalOutput")

    with tile.TileContext(nc) as tc:
        with (
            tc.tile_pool(name="cst", bufs=1) as cst,
            tc.tile_pool(name="sb", bufs=3) as pool,
            tc.tile_pool(name="ps", bufs=2, space="PSUM") as psp,
            tc.tile_pool(name="ps2", bufs=2, space="PSUM") as psp2,
        ):
            w2t = cst.tile([HID, NCLS * 2], F32)
            nc.sync.dma_start(out=w2t[:], in_=w2c[:])
            b2t = cst.tile([1, NCLS * 2], F32)
            nc.sync.dma_start(out=b2t[:], in_=b2c[:])
            ident = cst.tile([P, P], F32)
            make_identity(nc, ident[:])
            ones_t = cst.tile([1, P], F32)
            nc.vector.memset(ones_t[:], 1.0)

            for t in range(ntiles):
                rows = min(P, SH - t * P)
                sl = slice(t * P, t * P + rows)
                pt = [pool.tile([P, HID], F32, tag=f"pp{i}", name=f"pp{i}") for i in range(4)]
                for i in range(4):
                    nc.sync.dma_start(out=pt[i][:rows, :], in_=ps_[i][sl, :])
                s1t = pool.tile([P, HID], F32, tag="s1")
                nc.sync.dma_start(out=s1t[:rows, :], in_=s1[sl, :])
                dvt = pool.tile([P, 1], F32, tag="dv")
                nc.sync.dma_start(out=dvt[:rows, :], in_=dinv[sl, :])

                a01 = pool.tile([P, HID], F32, tag="a01")
                nc.vector.tensor_add(out=a01[:rows], in0=pt[0][:rows], in1=pt[1][:rows])
                a23 = pool.tile([P, HID], F32, tag="a23")
                nc.vector.tensor_add(out=a23[:rows], in0=pt[2][:rows], in1=pt[3][:rows])
                agg = pool.tile([P, HID], F32, tag="agg")
                nc.vector.tensor_add(out=agg[:rows], in0=a01[:rows], in1=a23[:rows])
                sc = pool.tile([P, HID], F32, tag="sc")
                nc.vector.tensor_scalar_mul(out=sc[:rows], in0=agg[:rows], scalar1=dvt[:rows, :])
                pre = pool.tile([P, HID], F32, tag="pre")
                nc.vector.tensor_add(out=pre[:rows], in0=sc[:rows], in1=s1t[:rows])
                out1 = pool.tile([P, HID], F32, tag="out1")
                if rows < P:
                    nc.vector.memset(out1[:], 0.0)
                nc.vector.tensor_scalar_max(out=out1[:rows], in0=pre[:rows], scalar1=0.0)
                # transpose out1 -> [HID, P]
                o1T_ps = psp.tile([HID, P], F32, tag="o1T")
                nc.tensor.transpose(out=o1T_ps[:], in_=out1[:, :], identity=ident[:])
                o1T = pool.tile([HID, P], F32, tag="o1Ts")
                nc.vector.tensor_copy(out=o1T[:], in_=o1T_ps[:])
                # [out1@W2 | out1@V2] + [0|b2]
                mm = psp2.tile([P, NCLS * 2], F32, tag="mm2")
                nc.tensor.matmul(out=mm[:rows, :], lhsT=o1T[:, :rows], rhs=w2t[:], start=True, stop=False)
                nc.tensor.matmul(out=mm[:rows, :], lhsT=ones_t[:, :rows], rhs=b2t[:], start=False, stop=True)
                h2t = pool.tile([P, FP], F32, tag="h2")
                nc.vector.memset(h2t[:], 0.0)
                nc.vector.tensor_scalar_mul(out=h2t[:rows, :NCLS], in0=mm[:rows, :NCLS], scalar1=dvt[:rows, :])
                nc.sync.dma_start(out=h2p[sl, :], in_=h2t[:rows, :])
                s2t = pool.tile([P, NCLS], F32, tag="s2")
                nc.vector.tensor_copy(out=s2t[:rows, :], in_=mm[:rows, NCLS:])
                nc.sync.dma_start(out=s2[sl, :], in_=s2t[:rows, :])
    nc.compile()
    return nc


def _build_k5():
    """out = log_softmax(relu(dinv * (q0+q1+q2+q3) + s2))"""
    ntiles = (SH + P - 1) // P
    nc = _new_nc()
    ps_ = [nc.dram_tensor(f"q{i}", [SH, NCLS], F32, kind="ExternalInput") for i in range(4)]
    s2 = nc.dram_tensor("s2", [SH, NCLS], F32, kind="ExternalInput")
    dinv = nc.dram_tensor("dinv", [SH, 1], F32, kind="ExternalInput")
    out = nc.dram_tensor("out", [SH, NCLS], F32, kind="ExternalOutput")

    with tile.TileContext(nc) as tc:
        with tc.tile_pool(name="sb", bufs=3) as pool:
            for t in range(ntiles):
                rows = min(P, SH - t * P)
                sl = slice(t * P, t * P + rows)
                pt = [pool.tile([P, NCLS], F32, tag=f"pp{i}", name=f"pp{i}") for i in range(4)]
                for i in range(4):
                    nc.sync.dma_start(out=pt[i][:rows, :], in_=ps_[i][sl, :])
                s2t = pool.tile([P, NCLS], F32, tag="s2")
                nc.sync.dma_start(out=s2t[:rows, :], in_=s2[sl, :])
                dvt = pool.tile([P, 1], F32, tag="dv")
                nc.sync.dma_start(out=dvt[:rows, :], in_=dinv[sl, :])

                a01 = pool.tile([P, NCLS], F32, tag="a01")
                nc.vector.tensor_add(out=a01[:rows], in0=pt[0][:rows], in1=pt[1][:rows])
                a23 = pool.tile([P, NCLS], F32, tag="a23")
                nc.vector.tensor_add(out=a23[:rows], in0=pt[2][:rows], in1=pt[3][:rows])
                agg = pool.tile([P, NCLS], F32, tag="agg")
                nc.vector.tensor_add(out=agg[:rows], in0=a01[:rows], in1=a23[:rows])
                sc = pool.tile([P, NCLS], F32, tag="sc")
                nc.vector.tensor_scalar_mul(out=sc[:rows], in0=agg[:rows], scalar1=dvt[:rows, :])
                pre = pool.tile([P, NCLS], F32, tag="pre")
                nc.vector.tensor_add(out=pre[:rows], in0=sc[:rows], in1=s2t[:rows])
                o2 = pool.tile([P, NCLS], F32, tag="o2")
                nc.vector.tensor_scalar_max(out=o2[:rows], in0=pre[:rows], scalar1=0.0)
                # log_softmax
                mx = pool.tile([P, 1], F32, tag="mx")
                nc.vector.reduce_max(out=mx[:rows], in_=o2[:rows], axis=mybir.AxisListType.X)
                tshift = pool.tile([P, NCLS], F32, tag="ts")
                nc.vector.tensor_scalar_sub(out=tshift[:rows], in0=o2[:rows], scalar1=mx[:rows, :])
                ex = pool.tile([P, NCLS], F32, tag="ex")
                nc.scalar.activation(
                    out=ex[:rows], in_=tshift[:rows],
                    func=mybir.ActivationFunctionType.Exp,
                )
                sm = pool.tile([P, 1], F32, tag="sm")
                nc.vector.reduce_sum(out=sm[:rows], in_=ex[:rows], axis=mybir.AxisListType.X)
                ls = pool.tile([P, 1], F32, tag="ls")
                nc.scalar.activation(
                    out=ls[:rows], in_=sm[:rows],
                    func=mybir.ActivationFunctionType.Ln,
                )
                res = pool.tile([P, NCLS], F32, tag="res")
                nc.vector.tensor_scalar_sub(out=res[:rows], in0=tshift[:rows], scalar1=ls[:rows, :])
                nc.sync.dma_start(out=out[sl, :], in_=res[:rows, :])
    nc.compile()
    return nc


# ================================================================ driver
def kernel(x, edge_index, edge_weight, W1, V1, b1, W2, V2, b2):
    x = np.asarray(x, np.float32)
    ew = np.asarray(edge_weight, np.float32)
    src = np.asarray(edge_index[0], np.int64)
    dst = np.asarray(edge_index[1], np.int64)
    W1 = np.asarray(W1, np.float32); V1 = np.asarray(V1, np.float32)
    b1 = np.asarray(b1, np.float32)
    W2 = np.asarray(W2, np.float32); V2 = np.asarray(V2, np.float32)
    b2 = np.asarray(b2, np.float32)

    # -------- L1 host prep: per-core (dst shard) degree slot grids
    core1 = dst // SH
    w1inputs = []
    grids1 = []
    for c in range(NC):
        m = core1 == c
        dl = dst[m] - c * SH
        counts = np.bincount(dl, minlength=SH)
        grids1.append((dl, ew[m], counts))
    nt1 = (SH + P - 1) // P
    d1prof = np.zeros(nt1, np.int64)
    for (dl, w, counts) in grids1:
        cpad = np.zeros(nt1 * P, np.int64)
        cpad[:SH] = counts
        d1prof = np.maximum(d1prof, cpad.reshape(nt1, P).max(axis=1))
    off1 = np.zeros(nt1, np.int64)
    np.cumsum(d1prof[:-1], out=off1[1:])
    tot1 = int(d1prof.sum())
    for (dl, w, counts) in grids1:
        order = np.argsort(dl, kind="stable")
        dls, ws = dl[order], w[order]
        starts = np.zeros(SH, np.int64)
        np.cumsum(counts[:-1], out=starts[1:])
        col = np.arange(len(dls)) - starts[dls]
        wgrid = np.zeros((P, tot1), np.float32)
        tile_of = dls // P
        wgrid[dls % P, off1[tile_of] + col] = ws
        w1inputs.append(wgrid)

    wcat = np.concatenate([W1, V1], axis=1)
    bcat = np.zeros((1, HID * 2), np.float32)
    bcat[0, HID:] = b1

    k1 = _build_k1(d1prof)
    in1 = [
        {
            "xt": np.ascontiguousarray(x[c * SH:(c + 1) * SH].T),
            "wsl": w1inputs[c],
            "wcat": wcat,
            "bcat": bcat,
        }
        for c in range(NC)
    ]
    r1 = _run(k1, in1)
    h1_full = np.concatenate([r1[c]["h1p"] for c in range(NC)], axis=0)  # [N, 64]
    s1_sh = [r1[c]["s1"] for c in range(NC)]
    dinv_sh = [r1[c]["dinv"] for c in range(NC)]

    # -------- L2/L4 host prep: 2D shard (dst half, src quarter)
    half = dst // HALF
    qtr = src // QTR
    agg_meta = []   # per core: (perm, row_s, col, idx_vals, w_vals)
    ntiles2 = HALF // P  # 50000/128 = 390.625 -> 391
    ntiles2 = (HALF + P - 1) // P
    counts_sorted_all = []
    for k in range(NC):
        d2, q = k // 4, k % 4
        m = (half == d2) & (qtr == q)
        dloc = dst[m] - d2 * HALF
        counts = np.bincount(dloc, minlength=HALF)
        perm, counts_sorted, row_s, col, order = _slot_grid(dloc, counts, HALF, None)
        idx_vals = (src[m] - q * QTR)[order].astype(np.int16)
        w_vals = ew[m][order].astype(np.float32)
        agg_meta.append((perm, row_s, col, idx_vals, w_vals))
        counts_sorted_all.append(counts_sorted)
    cs_pad = np.zeros((NC, ntiles2 * P), np.int64)
    for k in range(NC):
        cs_pad[k, :HALF] = counts_sorted_all[k]
    dprof2 = cs_pad.reshape(NC, ntiles2, P).max(axis=2).max(axis=0)

    sbs = _make_call_plan(dprof2)
    call_plan_flat = []
    for sb in sbs:
        for (c0, cn) in sb["calls"]:
            call_plan_flat.append((sb["c0_abs"] + c0, cn))

    agg_inputs = []
    for k in range(NC):
        perm, row_s, col, idx_vals, w_vals = agg_meta[k]
        ig, wgr, totc = _grid_inputs(row_s, col, ntiles2, dprof2, idx_vals, w_vals)
        idxw = _wrap_idx_calls(ig, call_plan_flat)
        agg_inputs.append({"idxw": idxw, "wg": wgr})

    k2 = _build_k2(dprof2, HID)
    in2 = []
    for k in range(NC):
        q = k % 4
        in2.append({
            "table": h1_full[q * QTR:(q + 1) * QTR],
            **agg_inputs[k],
        })
    r2 = _run(k2, in2)

    # un-permute partials, slice per L3 core
    def part_for(core, results, fcols):
        d2, o = core // 4, (core % 4) * SH
        outs = []
        for q in range(4):
            k = d2 * 4 + q
            perm = agg_meta[k][0]
            pr = results[k]["partial"][:HALF]  # rank order
            un = np.zeros((HALF, fcols), np.float32)
            un[perm] = pr
            outs.append(un[o:o + SH])
        return outs

    w2c = np.concatenate([W2, V2], axis=1)  # [48, 80]
    b2c = np.zeros((1, NCLS * 2), np.float32)
    b2c[0, NCLS:] = b2

    k3 = _build_k3()
    in3 = []
    for c in range(NC):
        p4 = part_for(c, r2, HID)
        in3.append({
            **{f"p{i}": p4[i] for i in range(4)},
            "s1": s1_sh[c], "dinv": dinv_sh[c], "w2c": w2c, "b2c": b2c,
        })
    r3 = _run(k3, in3)
    h2_full = np.concatenate([r3[c]["h2p"] for c in range(NC)], axis=0)
    s2_sh = [r3[c]["s2"] for c in range(NC)]

    # -------- L4: same aggregation with table = h2
    k4 = _build_k2(dprof2, NCLS)
    in4 = []
    for k in range(NC):
        q = k % 4
        in4.append({
            "table": h2_full[q * QTR:(q + 1) * QTR],
            **agg_inputs[k],
        })
    r4 = _run(k4, in4)

    k5 = _build_k5()
    in5 = []
    for c in range(NC):
        q4 = part_for(c, r4, NCLS)
        in5.append({
            **{f"q{i}": q4[i] for i in range(4)},
            "s2": s2_sh[c], "dinv": dinv_sh[c],
        })
    r5 = _run(k5, in5)
    out = np.concatenate([r5[c]["out"] for c in range(NC)], axis=0)
    return out.astype(np.float32)


# revision 11
# speedup vs baseline: 1.5073x; 1.5073x over previous
"""Trainium2 Bass kernel for the 2-layer ARMA GNN (nn_ARMA_30374008717356).

Self-contained: accepts FULL inputs, returns FULL output. Internally:
- 5 SPMD launches on 8 NeuronCores via run_bass_kernel_spmd.
- Math folding: norm = dinv[src]*w*dinv[dst] is folded as
  h~ = (x@W)*dinv (row scale at the table), per-edge scale w only, and a
  dinv[dst] row scale applied after aggregation. So
  agg[d] = dinv[d] * sum_e w_e * h~[src_e].
- Aggregation launches are 2D-sharded: core = (dst half, src quarter), so
  every dma_gather call's int16 indices fit a single 25000-row window.
  Per-core slot grids: dsts sorted by per-core edge count, tiles of 128
  rows x D_pad slot columns; pad slots point at row 0 with weight 0.
- Host between launches does only sharding / permutation / padding
  (index manipulation); all arithmetic happens on device.
"""
import os
import sys
import types
import contextlib
import ctypes

import numpy as np

sys.path.insert(0, "/opt/trn_rl_repo")

import concourse.bass as bass
import concourse.bacc as bacc
import concourse.mybir as mybir
import concourse.tile as tile
from concourse.bass_utils import run_bass_kernel_spmd
from concourse.masks import make_identity

# ---------------------------------------------------------------- constants
N = 100000
E = 1600000
F_IN, HID, NCLS = 64, 48, 40
NC = 8
SH = N // NC          # 12500 nodes per core (layer-compute shards)
HALF = N // 2         # 50000 dst half
QTR = N // 4          # 25000 src quarter window (int16-safe)
P = 128
FP = 64               # padded feature count -> 256B gather rows
GCALL_COLS = 7        # 7 cols * 128 rows = 896 idx per dma_gather call (ring-safe)
SB_COLS = 160         # superblock column budget (~41KB/partition fp32)

F32 = mybir.dt.float32
I16 = mybir.dt.int16

_TRACE = [False]      # set by test harness to collect exec times
_EXEC_NS = []


# ------------------------------------------------------------ axon NTFF shim
def _install_profile_shim():
    if "antenv.axon_hooks" in sys.modules:
        return
    try:
        import antenv
        from trn_agent_boot.trn_boot import _ntff_profile_via_ctypes
    except Exception:
        return
    hook_holder = {"h": None}
    mod = types.ModuleType("antenv.axon_hooks")
    mod.set_axon_ntff_profile_hook = lambda h: hook_holder.__setitem__("h", h)
    mod.get_axon_ntff_profile_hook = lambda: hook_holder["h"]
    sys.modules["antenv.axon_hooks"] = mod
    antenv.axon_hooks = mod
    try:
        h = _ntff_profile_via_ctypes("/opt/axon/libaxon_pjrt.so")
        if h is not None:
            mod.set_axon_ntff_profile_hook(h)
    except Exception:
        pass


_install_profile_shim()


def _run(nc, in_maps):
    trace = _TRACE[0]
    res = run_bass_kernel_spmd(nc, in_maps, core_ids=list(range(NC)), trace=trace)
    if trace:
        _EXEC_NS.append(res.exec_time_ns)
    return res.results


def _new_nc(nq=1):
    return bacc.Bacc(
        "TRN2",
        target_bir_lowering=False,
        debug=False,
        num_devices=NC,
        num_swdge_queues=nq,
    )


# ================================================================ host prep
def _slot_grid(dloc, order_key_counts, ndst, vals_list):
    """Build per-dst slot assignment for one core.

    dloc: local dst id per edge (int64, [M])
    order_key_counts: per-dst edge counts [ndst]
    vals_list: list of per-edge value arrays to place into grids
    Returns (perm, rank_counts_sorted, row, col) where row = rank of dst,
    col = slot index within dst, perm = dst ids sorted by count desc.
    """
    counts = order_key_counts
    perm = np.argsort(-counts, kind="stable")
    rank = np.empty(ndst, np.int64)
    rank[perm] = np.arange(ndst)
    row = rank[dloc]
    order = np.argsort(row, kind="stable")
    row_s = row[order]
    counts_sorted = counts[perm]
    starts = np.zeros(ndst, np.int64)
    np.cumsum(counts_sorted[:-1], out=starts[1:])
    col = np.arange(len(row_s)) - starts[row_s]
    return perm, counts_sorted, row_s, col, order


def _grid_inputs(row_s, col, ntiles, dprof, idx_vals, w_vals):
    """Assemble device inputs for one aggregation core.

    Returns (idx_blocks [128, totcols*? int16 16-wrapped per call],
             w_grid [128, totcols] f32, plan) -- plan computed separately.
    idx_vals/w_vals are in slot order (row_s, col).
    """
    totcols = int(np.sum(dprof))
    col_off = np.zeros(ntiles, np.int64)
    np.cumsum(dprof[:-1], out=col_off[1:])
    tile_of_row = row_s // P
    part = row_s % P
    gcol = col_off[tile_of_row] + col
    wg = np.zeros((P, totcols), np.float32)
    ig = np.zeros((P, totcols), np.int16)
    wg[part, gcol] = w_vals
    ig[part, gcol] = idx_vals
    return ig, wg, totcols


def _wrap_idx_calls(ig, call_plan):
    """ig: [128, totcols] int16 grid. call_plan: list of (c0, ncols).
    Returns [128, sum(ncols*8)] int16 ready for device (16-wrap + x8 tile)."""
    blocks = []
    for c0, ncols in call_plan:
        flat = ig[:, c0:c0 + ncols].T.ravel()          # i = col*128 + p
        b = flat.reshape(-1, 16).T                      # [16, n/16]
        blocks.append(np.tile(b, (8, 1)))               # [128, n/16]
    return np.concatenate(blocks, axis=1)


def _make_call_plan(dprof):
    """Split the flat column space into superblocks and gather calls.

    Returns list of superblocks; each is dict with tiles [(tile_idx, c0_rel,
    D)], ncols, calls [(c0_rel, ncols)], c0_abs."""
    sbs = []
    cur = {"tiles": [], "ncols": 0, "c0_abs": 0}
    c_abs = 0
    for t, d in enumerate(dprof):
        d = int(d)
        if d == 0:
            # still need output; attach to current superblock with D=0
            cur["tiles"].append((t, cur["ncols"], 0))
            continue
        if cur["ncols"] + d > SB_COLS and cur["ncols"] > 0:
            sbs.append(cur)
            cur = {"tiles": [], "ncols": 0, "c0_abs": c_abs}
        cur["tiles"].append((t, cur["ncols"], d))
        cur["ncols"] += d
        c_abs += d
    if cur["tiles"]:
        sbs.append(cur)
    for sb in sbs:
        calls = []
        c = 0
        while c < sb["ncols"]:
            n = min(GCALL_COLS, sb["ncols"] - c)
            calls.append((c, n))
            c += n
        sb["calls"] = calls
    return sbs


# ================================================================ kernels
def _build_k1(d1prof):
    """Per-core: deg -> dinv; h1 = (x@W1)*dinv (padded to 64); s1 = x@V1+b1.

    Inputs: xt [64, SHP], wsl [128, tot1], wcat [64, 96], bcat [1, 96].
    Outputs: h1p [SHP,64], s1 [SHP,48], dinv [SHP,1]. SHP = 98*128 (padded).
    """
    ntiles = (SH + P - 1) // P
    SHP = ntiles * P
    tot1 = int(np.sum(d1prof))
    off = np.zeros(ntiles, np.int64)
    np.cumsum(d1prof[:-1], out=off[1:])

    nc = _new_nc()
    xt = nc.dram_tensor("xt", [F_IN, SHP], F32, kind="ExternalInput")
    wsl = nc.dram_tensor("wsl", [P, max(tot1, 1)], F32, kind="ExternalInput")
    wcat = nc.dram_tensor("wcat", [F_IN, HID * 2], F32, kind="ExternalInput")
    bcat = nc.dram_tensor("bcat", [1, HID * 2], F32, kind="ExternalInput")
    h1p = nc.dram_tensor("h1p", [SHP, FP], F32, kind="ExternalOutput")
    s1 = nc.dram_tensor("s1", [SHP, HID], F32, kind="ExternalOutput")
    dinv_o = nc.dram_tensor("dinv", [SHP, 1], F32, kind="ExternalOutput")

    with tile.TileContext(nc) as tc:
        with (
            tc.tile_pool(name="cst", bufs=1) as cst,
            tc.tile_pool(name="big", bufs=1) as big,
            tc.tile_pool(name="sb", bufs=4) as pool,
            tc.tile_pool(name="ps", bufs=4, space="PSUM") as psp,
        ):
            xt_t = cst.tile([F_IN, SHP], F32)
            nc.sync.dma_start(out=xt_t[:], in_=xt[:])
            wsl_t = cst.tile([P, max(tot1, 1)], F32)
            nc.sync.dma_start(out=wsl_t[:], in_=wsl[:])
            wc_t = cst.tile([F_IN, HID * 2], F32)
            nc.sync.dma_start(out=wc_t[:], in_=wcat[:])
            bc_t = cst.tile([1, HID * 2], F32)
            nc.sync.dma_start(out=bc_t[:], in_=bcat[:])
            ones_t = cst.tile([1, P], F32)
            nc.vector.memset(ones_t[:], 1.0)

            deg_b = big.tile([P, ntiles], F32)
            h1_b = big.tile([P, ntiles, FP], F32)
            s1_b = big.tile([P, ntiles, HID], F32)
            nc.vector.memset(h1_b[:], 0.0)

            for t in range(ntiles):
                d1 = int(d1prof[t])
                if d1 > 0:
                    nc.vector.reduce_sum(
                        out=deg_b[:, t:t + 1], in_=wsl_t[:, off[t]:off[t] + d1],
                        axis=mybir.AxisListType.X,
                    )
                else:
                    nc.vector.memset(deg_b[:, t:t + 1], 0.0)
                ps = psp.tile([P, HID * 2], F32, tag="mm")
                nc.tensor.matmul(
                    out=ps[:], lhsT=xt_t[:, t * P:(t + 1) * P],
                    rhs=wc_t[:, :], start=True, stop=False,
                )
                nc.tensor.matmul(
                    out=ps[:], lhsT=ones_t[:, :],
                    rhs=bc_t[:, :], start=False, stop=True,
                )
                nc.vector.tensor_copy(out=h1_b[:, t, :HID], in_=ps[:, :HID])
                nc.vector.tensor_copy(out=s1_b[:, t, :], in_=ps[:, HID:])

            # wide dinv: mask * 1/sqrt(max(deg,eps))
            mask = pool.tile([P, ntiles], F32)
            nc.vector.tensor_scalar(
                out=mask[:], in0=deg_b[:], scalar1=0.0, scalar2=None,
                op0=mybir.AluOpType.is_gt,
            )
            degc = pool.tile([P, ntiles], F32)
            nc.vector.tensor_scalar_max(out=degc[:], in0=deg_b[:], scalar1=1e-30)
            sq = pool.tile([P, ntiles], F32)
            nc.scalar.activation(
                out=sq[:], in_=degc[:], func=mybir.ActivationFunctionType.Sqrt,
            )
            rs = pool.tile([P, ntiles], F32)
            nc.vector.reciprocal(out=rs[:], in_=sq[:])
            dinv_b = pool.tile([P, ntiles], F32)
            nc.vector.tensor_mul(out=dinv_b[:], in0=rs[:], in1=mask[:])
            nc.sync.dma_start(
                out=dinv_o[:].rearrange("(t p) one -> p (t one)", p=P),
                in_=dinv_b[:],
            )
            # h1 *= dinv (broadcast over features)
            nc.vector.tensor_mul(
                out=h1_b[:, :, :HID], in0=h1_b[:, :, :HID],
                in1=dinv_b[:].to_broadcast([P, ntiles, HID]),
            )
            nc.sync.dma_start(
                out=h1p[:].rearrange("(t p) f -> p t f", p=P), in_=h1_b[:]
            )
            nc.sync.dma_start(
                out=s1[:].rearrange("(t p) f -> p t f", p=P), in_=s1_b[:]
            )
    nc.compile()
    return nc


def _build_k2(dprof, fuse_cols):
    """Aggregation core: partial[r] = sum_slots w * table[idx].

    Inputs: table [QTR, FP], idxw (16-wrapped per call) [128, idx_cols],
    wg [128, totcols]. Output: partial [ntiles*128, fuse_cols].
    """
    ntiles = len(dprof)
    sbs = _make_call_plan(dprof)
    idx_cols = sum(8 * n for sb in sbs for (_, n) in sb["calls"])
    totcols = int(np.sum(dprof))

    nc = _new_nc(nq=4)
    table = nc.dram_tensor("table", [QTR, FP], F32, kind="ExternalInput")
    idxw = nc.dram_tensor("idxw", [P, max(idx_cols, 1)], I16, kind="ExternalInput")
    wg = nc.dram_tensor("wg", [P, max(totcols, 1)], F32, kind="ExternalInput")
    partial = nc.dram_tensor("partial", [ntiles * P, fuse_cols], F32, kind="ExternalOutput")

    qn = [0]

    with tile.TileContext(nc) as tc:
        with (
            tc.tile_pool(name="cst", bufs=1) as cst,
            tc.tile_pool(name="gat", bufs=2) as gat,
            tc.tile_pool(name="wrk", bufs=3) as wrk,
        ):
            idx_t = cst.tile([P, max(idx_cols, 1)], I16)
            nc.sync.dma_start(out=idx_t[:], in_=idxw[:])
            wg_t = cst.tile([P, max(totcols, 1)], F32)
            nc.sync.dma_start(out=wg_t[:], in_=wg[:])

            iwoff = 0
            for sb in sbs:
                ncols = sb["ncols"]
                if ncols > 0:
                    g = gat.tile([P, ncols, FP], F32, tag="g")
                    for (c0, cn) in sb["calls"]:
                        nidx = P * cn
                        nc.gpsimd.dma_gather(
                            g[:, c0:c0 + cn, :],
                            table[:],
                            idx_t[:, iwoff:iwoff + 8 * cn],
                            nidx,
                            nidx,
                            FP,
                            queue_num=qn[0] % 4,
                        )
                        qn[0] += 1
                        iwoff += 8 * cn
                for (t, c0r, d) in sb["tiles"]:
                    out_t = wrk.tile([P, fuse_cols], F32, tag="out")
                    if d == 0:
                        nc.vector.memset(out_t[:], 0.0)
                    else:
                        c_abs = sb["c0_abs"] + c0r
                        sc = wrk.tile([P, d, fuse_cols], F32, tag="sc")
                        nc.vector.tensor_mul(
                            out=sc[:],
                            in0=g[:, c0r:c0r + d, :fuse_cols],
                            in1=wg_t[:, c_abs:c_abs + d]
                                .to_broadcast([P, d, fuse_cols]),
                        )
                        nc.vector.reduce_sum(
                            out=out_t[:],
                            in_=sc[:].rearrange("p d f -> p f d"),
                            axis=mybir.AxisListType.X,
                        )
                    nc.sync.dma_start(
                        out=partial[t * P:(t + 1) * P, :], in_=out_t[:]
                    )
    nc.compile()
    return nc


def _build_k3():
    """Combine 4 partials, finish layer 1, produce layer-2 table inputs.

    All [SHP, *] padded to 98*128 rows.
    """
    ntiles = (SH + P - 1) // P
    SHP = ntiles * P
    nc = _new_nc()
    ps_ = [nc.dram_tensor(f"p{i}", [SHP, HID], F32, kind="ExternalInput") for i in range(4)]
    s1 = nc.dram_tensor("s1", [SHP, HID], F32, kind="ExternalInput")
    dinv = nc.dram_tensor("dinv", [SHP, 1], F32, kind="ExternalInput")
    w2c = nc.dram_tensor("w2c", [HID, NCLS * 2], F32, kind="ExternalInput")
    b2c = nc.dram_tensor("b2c", [1, NCLS * 2], F32, kind="ExternalInput")
    h2p = nc.dram_tensor("h2p", [SHP, FP], F32, kind="ExternalOutput")
    s2 = nc.dram_tensor("s2", [SHP, NCLS], F32, kind="ExternalOutput")

    with tile.TileContext(nc) as tc:
        with (
            tc.tile_pool(name="cst", bufs=1) as cst,
            tc.tile_pool(name="big", bufs=1) as big,
            tc.tile_pool(name="sb", bufs=1) as pool,
            tc.tile_pool(name="tp", bufs=4) as tpool,
            tc.tile_pool(name="ps", bufs=4, space="PSUM") as psp,
            tc.tile_pool(name="ps2", bufs=4, space="PSUM") as psp2,
        ):
            w2t = cst.tile([HID, NCLS * 2], F32)
            nc.sync.dma_start(out=w2t[:], in_=w2c[:])
            b2t = cst.tile([1, NCLS * 2], F32)
            nc.sync.dma_start(out=b2t[:], in_=b2c[:])
            ident = cst.tile([P, P], F32)
            make_identity(nc, ident[:])
            ones_t = cst.tile([1, P], F32)
            nc.vector.memset(ones_t[:], 1.0)

            acc = cst.tile([P, ntiles, HID], F32)
            nc.sync.dma_start(out=acc[:], in_=ps_[0][:].rearrange("(t p) f -> p t f", p=P))
            for i in range(1, 4):
                stg = pool.tile([P, ntiles, HID], F32, tag="stg", name="stg")
                nc.sync.dma_start(out=stg[:], in_=ps_[i][:].rearrange("(t p) f -> p t f", p=P))
                nc.vector.tensor_add(out=acc[:], in0=acc[:], in1=stg[:])
            s1t = cst.tile([P, ntiles, HID], F32)
            nc.sync.dma_start(out=s1t[:], in_=s1[:].rearrange("(t p) f -> p t f", p=P))
            dvt = cst.tile([P, ntiles], F32)
            nc.sync.dma_start(out=dvt[:], in_=dinv[:].rearrange("(t p) one -> p (t one)", p=P))

            out1 = pool.tile([P, ntiles, HID], F32)
            nc.vector.tensor_mul(
                out=out1[:], in0=acc[:], in1=dvt[:].to_broadcast([P, ntiles, HID])
            )
            nc.vector.tensor_add(out=out1[:], in0=out1[:], in1=s1t[:])
            nc.vector.tensor_scalar_max(out=out1[:], in0=out1[:], scalar1=0.0)

            mm_b = big.tile([P, ntiles, NCLS * 2], F32)
            for t in range(ntiles):
                o1T_ps = psp.tile([HID, P], F32, tag="o1T")
                nc.tensor.transpose(out=o1T_ps[:], in_=out1[:, t, :], identity=ident[:])
                o1T = tpool.tile([HID, P], F32, tag="o1Ts")
                nc.vector.tensor_copy(out=o1T[:], in_=o1T_ps[:])
                mm = psp2.tile([P, NCLS * 2], F32, tag="mm2")
                nc.tensor.matmul(out=mm[:], lhsT=o1T[:], rhs=w2t[:], start=True, stop=False)
                nc.tensor.matmul(out=mm[:], lhsT=ones_t[:], rhs=b2t[:], start=False, stop=True)
                nc.vector.tensor_copy(out=mm_b[:, t, :], in_=mm[:])

            h2_b = big.tile([P, ntiles, FP], F32)
            nc.vector.memset(h2_b[:], 0.0)
            nc.vector.tensor_mul(
                out=h2_b[:, :, :NCLS], in0=mm_b[:, :, :NCLS],
                in1=dvt[:].to_broadcast([P, ntiles, NCLS]),
            )
            nc.sync.dma_start(out=h2p[:].rearrange("(t p) f -> p t f", p=P), in_=h2_b[:])
            s2_b = big.tile([P, ntiles, NCLS], F32)
            nc.vector.tensor_copy(out=s2_b[:], in_=mm_b[:, :, NCLS:])
            nc.sync.dma_start(out=s2[:].rearrange("(t p) f -> p t f", p=P), in_=s2_b[:])
    nc.compile()
    return nc


def _build_k5():
    """out = log_softmax(relu(dinv * (q0+q1+q2+q3) + s2)); padded rows."""
    ntiles = (SH + P - 1) // P
    SHP = ntiles * P
    nc = _new_nc()
    ps_ = [nc.dram_tensor(f"q{i}", [SHP, NCLS], F32, kind="ExternalInput") for i in range(4)]
    s2 = nc.dram_tensor("s2", [SHP, NCLS], F32, kind="ExternalInput")
    dinv = nc.dram_tensor("dinv", [SHP, 1], F32, kind="ExternalInput")
    out = nc.dram_tensor("out", [SHP, NCLS], F32, kind="ExternalOutput")

    with tile.TileContext(nc) as tc:
        with (
            tc.tile_pool(name="cst", bufs=1) as cst,
            tc.tile_pool(name="sb", bufs=1) as pool,
        ):
            acc = cst.tile([P, ntiles, NCLS], F32)
            nc.sync.dma_start(out=acc[:], in_=ps_[0][:].rearrange("(t p) f -> p t f", p=P))
            for i in range(1, 4):
                stg = pool.tile([P, ntiles, NCLS], F32, tag="stg", name="stg")
                nc.sync.dma_start(out=stg[:], in_=ps_[i][:].rearrange("(t p) f -> p t f", p=P))
                nc.vector.tensor_add(out=acc[:], in0=acc[:], in1=stg[:])
            s2t = cst.tile([P, ntiles, NCLS], F32)
            nc.sync.dma_start(out=s2t[:], in_=s2[:].rearrange("(t p) f -> p t f", p=P))
            dvt = cst.tile([P, ntiles], F32)
            nc.sync.dma_start(out=dvt[:], in_=dinv[:].rearrange("(t p) one -> p (t one)", p=P))

            o2 = pool.tile([P, ntiles, NCLS], F32)
            nc.vector.tensor_mul(
                out=o2[:], in0=acc[:], in1=dvt[:].to_broadcast([P, ntiles, NCLS])
            )
            nc.vector.tensor_add(out=o2[:], in0=o2[:], in1=s2t[:])
            nc.vector.tensor_scalar_max(out=o2[:], in0=o2[:], scalar1=0.0)

            mx = pool.tile([P, ntiles], F32)
            nc.vector.reduce_max(out=mx[:], in_=o2[:], axis=mybir.AxisListType.X)
            ts = pool.tile([P, ntiles, NCLS], F32)
            nc.vector.tensor_tensor(
                out=ts[:], in0=o2[:], in1=mx[:].to_broadcast([P, ntiles, NCLS]),
                op=mybir.AluOpType.subtract,
            )
            ex = pool.tile([P, ntiles, NCLS], F32)
            nc.scalar.activation(
                out=ex[:], in_=ts[:], func=mybir.ActivationFunctionType.Exp,
            )
            sm = pool.tile([P, ntiles], F32)
            nc.vector.reduce_sum(out=sm[:], in_=ex[:], axis=mybir.AxisListType.X)
            ls = pool.tile([P, ntiles], F32)
            nc.scalar.activation(
                out=ls[:], in_=sm[:], func=mybir.ActivationFunctionType.Ln,
            )
            nc.vector.tensor_tensor(
                out=ex[:], in0=ts[:], in1=ls[:].to_broadcast([P, ntiles, NCLS]),
                op=mybir.AluOpType.subtract,
            )
            nc.sync.dma_start(out=out[:].rearrange("(t p) f -> p t f", p=P), in_=ex[:])
    nc.compile()
    return nc


# ================================================================ driver
def kernel(x, edge_index, edge_weight, W1, V1, b1, W2, V2, b2):
    x = np.asarray(x, np.float32)
    ew = np.asarray(edge_weight, np.float32)
    src = np.asarray(edge_index[0], np.int64)
    dst = np.asarray(edge_index[1], np.int64)
    W1 = np.asarray(W1, np.float32); V1 = np.asarray(V1, np.float32)
    b1 = np.asarray(b1, np.float32)
    W2 = np.asarray(W2, np.float32); V2 = np.asarray(V2, np.float32)
    b2 = np.asarray(b2, np.float32)

    # -------- L1 host prep: per-core (dst shard) degree slot grids
    core1 = dst // SH
    w1inputs = []
    grids1 = []
    for c in range(NC):
        m = core1 == c
        dl = dst[m] - c * SH
        counts = np.bincount(dl, minlength=SH)
        grids1.append((dl, ew[m], counts))
    nt1 = (SH + P - 1) // P
    d1prof = np.zeros(nt1, np.int64)
    for (dl, w, counts) in grids1:
        cpad = np.zeros(nt1 * P, np.int64)
        cpad[:SH] = counts
        d1prof = np.maximum(d1prof, cpad.reshape(nt1, P).max(axis=1))
    off1 = np.zeros(nt1, np.int64)
    np.cumsum(d1prof[:-1], out=off1[1:])
    tot1 = int(d1prof.sum())
    for (dl, w, counts) in grids1:
        order = np.argsort(dl, kind="stable")
        dls, ws = dl[order], w[order]
        starts = np.zeros(SH, np.int64)
        np.cumsum(counts[:-1], out=starts[1:])
        col = np.arange(len(dls)) - starts[dls]
        wgrid = np.zeros((P, tot1), np.float32)
        tile_of = dls // P
        wgrid[dls % P, off1[tile_of] + col] = ws
        w1inputs.append(wgrid)

    wcat = np.concatenate([W1, V1], axis=1)
    bcat = np.zeros((1, HID * 2), np.float32)
    bcat[0, HID:] = b1

    nt1 = (SH + P - 1) // P
    SHP = nt1 * P
    def padrows(a, rows=SHP):
        out = np.zeros((rows,) + a.shape[1:], a.dtype)
        out[:a.shape[0]] = a
        return out

    k1 = _build_k1(d1prof)
    in1 = [
        {
            "xt": padrows(np.ascontiguousarray(x[c * SH:(c + 1) * SH]), SHP).T.copy(),
            "wsl": w1inputs[c],
            "wcat": wcat,
            "bcat": bcat,
        }
        for c in range(NC)
    ]
    r1 = _run(k1, in1)
    h1_full = np.concatenate([r1[c]["h1p"][:SH] for c in range(NC)], axis=0)  # [N, 64]
    s1_sh = [r1[c]["s1"] for c in range(NC)]          # [SHP, 48] padded
    dinv_sh = [r1[c]["dinv"] for c in range(NC)]      # [SHP, 1] padded

    # -------- L2/L4 host prep: 2D shard (dst half, src quarter)
    half = dst // HALF
    qtr = src // QTR
    agg_meta = []   # per core: (perm, row_s, col, idx_vals, w_vals)
    ntiles2 = HALF // P  # 50000/128 = 390.625 -> 391
    ntiles2 = (HALF + P - 1) // P
    counts_sorted_all = []
    for k in range(NC):
        d2, q = k // 4, k % 4
        m = (half == d2) & (qtr == q)
        dloc = dst[m] - d2 * HALF
        counts = np.bincount(dloc, minlength=HALF)
        perm, counts_sorted, row_s, col, order = _slot_grid(dloc, counts, HALF, None)
        idx_vals = (src[m] - q * QTR)[order].astype(np.int16)
        w_vals = ew[m][order].astype(np.float32)
        agg_meta.append((perm, row_s, col, idx_vals, w_vals))
        counts_sorted_all.append(counts_sorted)
    cs_pad = np.zeros((NC, ntiles2 * P), np.int64)
    for k in range(NC):
        cs_pad[k, :HALF] = counts_sorted_all[k]
    dprof2 = cs_pad.reshape(NC, ntiles2, P).max(axis=2).max(axis=0)

    sbs = _make_call_plan(dprof2)
    call_plan_flat = []
    for sb in sbs:
        for (c0, cn) in sb["calls"]:
            call_plan_flat.append((sb["c0_abs"] + c0, cn))

    agg_inputs = []
    for k in range(NC):
        perm, row_s, col, idx_vals, w_vals = agg_meta[k]
        ig, wgr, totc = _grid_inputs(row_s, col, ntiles2, dprof2, idx_vals, w_vals)
        idxw = _wrap_idx_calls(ig, call_plan_flat)
        agg_inputs.append({"idxw": idxw, "wg": wgr})

    k2 = _build_k2(dprof2, HID)
    in2 = []
    for k in range(NC):
        q = k % 4
        in2.append({
            "table": h1_full[q * QTR:(q + 1) * QTR],
            **agg_inputs[k],
        })
    r2 = _run(k2, in2)

    # un-permute partials, slice per L3 core
    def part_for(core, results, fcols):
        d2, o = core // 4, (core % 4) * SH
        outs = []
        for q in range(4):
            k = d2 * 4 + q
            perm = agg_meta[k][0]
            pr = results[k]["partial"][:HALF]  # rank order
            un = np.zeros((HALF, fcols), np.float32)
            un[perm] = pr
            outs.append(un[o:o + SH])
        return outs

    w2c = np.concatenate([W2, V2], axis=1)  # [48, 80]
    b2c = np.zeros((1, NCLS * 2), np.float32)
    b2c[0, NCLS:] = b2

    k3 = _build_k3()
    in3 = []
    for c in range(NC):
        p4 = part_for(c, r2, HID)
        in3.append({
            **{f"p{i}": padrows(p4[i]) for i in range(4)},
            "s1": s1_sh[c], "dinv": dinv_sh[c], "w2c": w2c, "b2c": b2c,
        })
    r3 = _run(k3, in3)
    h2_full = np.concatenate([r3[c]["h2p"][:SH] for c in range(NC)], axis=0)
    s2_sh = [r3[c]["s2"] for c in range(NC)]

    # -------- L4: same aggregation with table = h2
    k4 = _build_k2(dprof2, NCLS)
    in4 = []
    for k in range(NC):
        q = k % 4
        in4.append({
            "table": h2_full[q * QTR:(q + 1) * QTR],
            **agg_inputs[k],
        })
    r4 = _run(k4, in4)

    k5 = _build_k5()
    in5 = []
    for c in range(NC):
        q4 = part_for(c, r4, NCLS)
        in5.append({
            **{f"q{i}": padrows(q4[i]) for i in range(4)},
            "s2": s2_sh[c], "dinv": dinv_sh[c],
        })
    r5 = _run(k5, in5)
    out = np.concatenate([r5[c]["out"][:SH] for c in range(NC)], axis=0)
    return out.astype(np.float32)
